# revision 7
# baseline (speedup 1.0000x reference)
"""Trainium2 Bass kernel for the ContrastiveLearningModule loss.

Math (mirrors the reference):
  P = l2norm(relu(E @ W1.T + b1) @ W2.T + b2)  rowwise over [T,V,L,N,D]
  for each node type t, anchors idx[t][v,l,:]:
    pos  = sum_{(x,y) != (v,l)} exp(z . P[t,x,y,id]/TEMP)
    negi = sum_{s' != s}        exp(z . z_{s'}   /TEMP)
    negc = sum_{o,k}            exp(z . P[o,v,l,nid]/TEMP)
    loss = log(pos+negi+negc) - log(pos);  out = sum(loss)/1440

Key optimization: only the gathered rows (~10.8k of 96k) are ever projected.
The host shards by gathering each core's rows (3 of the 24 (t,v,l) anchor
groups per core, padded to a uniform shape so all 8 cores run one SPMD
program), pre-transposed to [D, cols] so the on-device matmuls need no
transposes.  Each core returns a partial loss sum; the host combines.
"""

import sys

import numpy as np

sys.path.insert(0, "/opt/trn_rl_repo")

import concourse.bacc as bacc
import concourse.bass as bass
import concourse.mybir as mybir
import concourse.tile as tile
from concourse.bass_utils import run_bass_kernel_spmd

F32 = mybir.dt.float32
AF = mybir.ActivationFunctionType

# Problem constants (hardcoded per harness contract).
T, V, L, N, D = 4, 2, 3, 4000, 256
TEMP = 0.5
S = 100          # padded anchors per group (prio groups are exactly 100)
KPAD = 50        # padded cross-negatives per other-type (prio exactly 50)
NK = 3 * KPAD    # 150 cross-negative columns per group
XY = V * L       # 6 (view, layer) slabs
GCOLS = XY * S   # 600 gathered positive columns per group
SLOT = 768       # column stride per group slot (600 + 150 + 18 pad)
NSLOT = 3        # groups per core
NCOL = SLOT * NSLOT  # 2304 packed columns per core
NCORES = 8
COUNT = 1440.0   # total anchor count in the reference loss
NEG_BIG = -15000.0  # additive mask; exp(2*(sims+NEG_BIG)) underflows to 0
COL_BLOCKS = [(0, 512), (512, 512), (1024, 512), (1536, 512), (2048, 256)]

_CACHE = {}


def _emit_body(nc, tc, consts, dram, rep):
    """One full loss computation: projection + per-slot similarities."""
    w1, w2, bb, ones_col, ones_row, eps = consts
    xt_d, mi_d, mc_d, ms_d, out_d = dram
    r = f"r{rep}"

    with (
        tc.tile_pool(name=f"phat{r}", bufs=1) as ppool,
    ):
        # P-hat: normalized projections, [d, col] layout, d split in 2.
        ph = [ppool.tile([128, NCOL], F32, name=f"ph{j}{r}", tag=f"ph{j}")
              for j in (0, 1)]

        # ---- Projection + normalization, 512-column blocks ----
        with (
            tc.tile_pool(name=f"xin{r}", bufs=3) as xpool,
            tc.tile_pool(name=f"work{r}", bufs=2) as wpool,
            tc.tile_pool(name=f"psmm{r}", bufs=4, space=bass.MemorySpace.PSUM) as psmm,
            tc.tile_pool(name=f"psnb{r}", bufs=2, space=bass.MemorySpace.PSUM) as psnb,
        ):
            for c0, w in COL_BLOCKS:
                xs = []
                for k in (0, 1):
                    x = xpool.tile([128, w], F32, name=f"x{k}", tag=f"x{k}")
                    nc.sync.dma_start(x[:], xt_d[k * 128:(k + 1) * 128, c0:c0 + w])
                    xs.append(x)
                hs = []
                for jt in (0, 1):
                    psh = psmm.tile([128, w], F32, name="ps", tag="ps")
                    nc.tensor.matmul(psh[:], w1[0][:, jt * 128:(jt + 1) * 128],
                                     xs[0][:], start=True, stop=False)
                    nc.tensor.matmul(psh[:], w1[1][:, jt * 128:(jt + 1) * 128],
                                     xs[1][:], start=False, stop=True)
                    h = wpool.tile([128, w], F32, name=f"h{jt}", tag=f"h{jt}")
                    nc.scalar.activation(h[:], psh[:], AF.Relu, bias=bb[:, jt:jt + 1])
                    hs.append(h)
                zs = []
                sqs = []
                for jt in (0, 1):
                    psz = psmm.tile([128, w], F32, name="ps", tag="ps")
                    nc.tensor.matmul(psz[:], w2[0][:, jt * 128:(jt + 1) * 128],
                                     hs[0][:], start=True, stop=False)
                    nc.tensor.matmul(psz[:], w2[1][:, jt * 128:(jt + 1) * 128],
                                     hs[1][:], start=False, stop=True)
                    z = wpool.tile([128, w], F32, name=f"z{jt}", tag=f"z{jt}")
                    nc.scalar.activation(z[:], psz[:], AF.Identity,
                                         bias=bb[:, 2 + jt:3 + jt])
                    zs.append(z)
                    sq = wpool.tile([128, w], F32, name=f"sq{jt}", tag=f"sq{jt}")
                    nc.vector.tensor_mul(sq[:], z[:], z[:])
                    sqs.append(sq)
                # ||z||^2 per column via ones-matmul partition reduction
                psn = psnb.tile([1, w], F32, name="psn", tag="psn")
                nc.tensor.matmul(psn[:], ones_col[:], sqs[0][:], start=True, stop=False)
                nc.tensor.matmul(psn[:], ones_col[:], sqs[1][:], start=False, stop=True)
                nrm = wpool.tile([1, w], F32, name="nrm", tag="nrm")
                nc.scalar.activation(nrm[:], psn[:], AF.Sqrt, bias=eps[:])
                rn = wpool.tile([1, w], F32, name="rn", tag="rn")
                nc.vector.reciprocal(rn[:], nrm[:])
                # broadcast 1/||z|| across partitions via rank-1 matmul
                psb = psnb.tile([128, w], F32, name="psb", tag="psb")
                nc.tensor.matmul(psb[:], ones_row[:], rn[:], start=True, stop=True)
                for jt in (0, 1):
                    nc.vector.tensor_mul(ph[jt][:, c0:c0 + w], zs[jt][:], psb[:])

        # ---- Per-slot similarities + loss ----
        with (
            tc.tile_pool(name=f"mpool{r}", bufs=1) as mpool,
            tc.tile_pool(name=f"sbs{r}", bufs=2) as spool,
            tc.tile_pool(name=f"pss{r}", bufs=2, space=bass.MemorySpace.PSUM) as pssim,
            tc.tile_pool(name=f"pst{r}", bufs=1, space=bass.MemorySpace.PSUM) as pstot,
        ):
            mi = mpool.tile([S, NSLOT * S], F32, name="mi", tag="mi")
            nc.sync.dma_start(mi[:], mi_d[:])
            mcm = mpool.tile([S, NSLOT * NK], F32, name="mc", tag="mc")
            nc.sync.dma_start(mcm[:], mc_d[:])
            ms = mpool.tile([S, NSLOT], F32, name="ms", tag="ms")
            nc.sync.dma_start(ms[:], ms_d[:])

            tot = pstot.tile([1, NSLOT], F32, name="tot", tag="tot")
            for s in range(NSLOT):
                o = s * SLOT
                # within-type sims: Z^T Z  [S, S]
                pin = pssim.tile([S, S], F32, name="pin", tag="pin")
                nc.tensor.matmul(pin[:], ph[0][:, o:o + S], ph[0][:, o:o + S],
                                 start=True, stop=False)
                nc.tensor.matmul(pin[:], ph[1][:, o:o + S], ph[1][:, o:o + S],
                                 start=False, stop=True)
                mski = spool.tile([S, S], F32, name="mski", tag="mski")
                nc.vector.tensor_add(mski[:], pin[:], mi[:, s * S:(s + 1) * S])
                ein = spool.tile([S, S], F32, name="ein", tag="ein")
                negin = spool.tile([S, 1], F32, name="negin", tag="negin")
                nc.scalar.activation(ein[:], mski[:], AF.Exp, scale=2.0,
                                     accum_out=negin[:])
                # cross-type sims: Z^T Nmat  [S, NK]
                pc = pssim.tile([S, NK], F32, name="pc", tag="pc")
                nc.tensor.matmul(pc[:], ph[0][:, o:o + S],
                                 ph[0][:, o + GCOLS:o + GCOLS + NK],
                                 start=True, stop=False)
                nc.tensor.matmul(pc[:], ph[1][:, o:o + S],
                                 ph[1][:, o + GCOLS:o + GCOLS + NK],
                                 start=False, stop=True)
                mskc = spool.tile([S, NK], F32, name="mskc", tag="mskc")
                nc.vector.tensor_add(mskc[:], pc[:], mcm[:, s * NK:(s + 1) * NK])
                ec = spool.tile([S, NK], F32, name="ec", tag="ec")
                negc = spool.tile([S, 1], F32, name="negc", tag="negc")
                nc.scalar.activation(ec[:], mskc[:], AF.Exp, scale=2.0,
                                     accum_out=negc[:])
                # positives: per-anchor dot with same node at other (x,y)
                ppos = pssim.tile([S, XY - 1], F32, name="ppos", tag="ppos")
                for xy in range(1, XY):
                    for jt in (0, 1):
                        pr = spool.tile([128, S], F32, name="pr", tag="pr")
                        nc.vector.tensor_mul(pr[:], ph[jt][:, o + xy * S:o + (xy + 1) * S],
                                             ph[jt][:, o:o + S])
                        nc.tensor.matmul(ppos[:, xy - 1:xy], pr[:], ones_col[:],
                                         start=(jt == 0), stop=(jt == 1))
                epos = spool.tile([S, XY - 1], F32, name="epos", tag="epos")
                pos = spool.tile([S, 1], F32, name="pos", tag="pos")
                nc.scalar.activation(epos[:], ppos[:], AF.Exp, scale=2.0,
                                     accum_out=pos[:])
                # loss_s = ln(pos+neg) - ln(pos), then mask+sum via matmul
                neg = spool.tile([S, 1], F32, name="neg", tag="neg")
                nc.vector.tensor_add(neg[:], negin[:], negc[:])
                den = spool.tile([S, 1], F32, name="den", tag="den")
                nc.vector.tensor_add(den[:], neg[:], pos[:])
                lnden = spool.tile([S, 1], F32, name="lnden", tag="lnden")
                nc.scalar.activation(lnden[:], den[:], AF.Ln)
                lnpos = spool.tile([S, 1], F32, name="lnpos", tag="lnpos")
                nc.scalar.activation(lnpos[:], pos[:], AF.Ln)
                lossv = spool.tile([S, 1], F32, name="lossv", tag="lossv")
                nc.vector.tensor_sub(lossv[:], lnden[:], lnpos[:])
                nc.tensor.matmul(tot[:, s:s + 1], lossv[:], ms[:, s:s + 1],
                                 start=True, stop=True)
            osb = spool.tile([1, 1], F32, name="osb", tag="osb")
            nc.vector.reduce_sum(osb[:], tot[:], axis=mybir.AxisListType.X)
            nc.sync.dma_start(out_d[:], osb[:])


def _build_nc(reps=1):
    nc = bacc.Bacc("TRN2", target_bir_lowering=False, debug=False)

    xt_d = nc.dram_tensor("xt", [D, NCOL], F32, kind="ExternalInput")
    w1t_d = nc.dram_tensor("w1t", [D, D], F32, kind="ExternalInput")
    w2t_d = nc.dram_tensor("w2t", [D, D], F32, kind="ExternalInput")
    bb_d = nc.dram_tensor("bb", [128, 4], F32, kind="ExternalInput")
    mi_d = nc.dram_tensor("mi", [S, NSLOT * S], F32, kind="ExternalInput")
    mc_d = nc.dram_tensor("mc", [S, NSLOT * NK], F32, kind="ExternalInput")
    ms_d = nc.dram_tensor("ms", [S, NSLOT], F32, kind="ExternalInput")
    out_d = nc.dram_tensor("out", [1, 1], F32, kind="ExternalOutput")

    with tile.TileContext(nc) as tc:
        with tc.tile_pool(name="const", bufs=1) as cpool:
            w1 = [cpool.tile([128, D], F32, name=f"w1_{k}", tag=f"w1_{k}")
                  for k in (0, 1)]
            w2 = [cpool.tile([128, D], F32, name=f"w2_{k}", tag=f"w2_{k}")
                  for k in (0, 1)]
            for k in (0, 1):
                nc.sync.dma_start(w1[k][:], w1t_d[k * 128:(k + 1) * 128, :])
                nc.sync.dma_start(w2[k][:], w2t_d[k * 128:(k + 1) * 128, :])
            bb = cpool.tile([128, 4], F32, name="bb", tag="bb")
            nc.sync.dma_start(bb[:], bb_d[:])
            ones_col = cpool.tile([128, 1], F32, name="ones_col", tag="ones_col")
            nc.vector.memset(ones_col[:], 1.0)
            ones_row = cpool.tile([1, 128], F32, name="ones_row", tag="ones_row")
            nc.vector.memset(ones_row[:], 1.0)
            eps = cpool.tile([1, 1], F32, name="eps", tag="eps")
            nc.vector.memset(eps[:], 1e-24)

            consts = (w1, w2, bb, ones_col, ones_row, eps)
            dram = (xt_d, mi_d, mc_d, ms_d, out_d)
            for rep in range(reps):
                _emit_body(nc, tc, consts, dram, rep)

    nc.compile()
    return nc


def _get_nc(reps=1):
    key = ("nc", reps)
    if key not in _CACHE:
        _CACHE[key] = _build_nc(reps)
    return _CACHE[key]


def _groups():
    gs = [(t, v, l) for t in range(T) for v in range(V) for l in range(L)]
    return [[gs[c], gs[c + NCORES], gs[c + 2 * NCORES]] for c in range(NCORES)]


def make_in_maps(node_embeddings, W1, b1, W2, b2, idx_prio, idx_rest,
                 neg_idx_prio, neg_idx_rest):
    E = np.asarray(node_embeddings, dtype=np.float32)
    W1 = np.asarray(W1, dtype=np.float32)
    b1 = np.asarray(b1, dtype=np.float32)
    W2 = np.asarray(W2, dtype=np.float32)
    b2 = np.asarray(b2, dtype=np.float32)
    idxp = np.asarray(idx_prio)
    idxr = np.asarray(idx_rest)
    nidxp = np.asarray(neg_idx_prio)
    nidxr = np.asarray(neg_idx_rest)

    w1t = np.ascontiguousarray(W1.T)
    w2t = np.ascontiguousarray(W2.T)
    bbm = np.stack([b1[:128], b1[128:], b2[:128], b2[128:]], axis=1)
    bbm = np.ascontiguousarray(bbm, dtype=np.float32)

    in_maps = []
    for gs in _groups():
        X = np.empty((NCOL, D), np.float32)
        MI = np.full((S, NSLOT * S), NEG_BIG, np.float32)
        MC = np.full((S, NSLOT * NK), NEG_BIG, np.float32)
        MS = np.zeros((S, NSLOT), np.float32)
        for si, (t, v, l) in enumerate(gs):
            if t < 2:
                idx, nid, Sr, Kr = idxp[t], nidxp[t], 100, 50
            else:
                idx, nid, Sr, Kr = idxr[t - 2], nidxr[t - 2], 20, 10
            ids = np.asarray(idx[v, l])
            ids_p = np.concatenate([ids, np.full(S - Sr, ids[0], ids.dtype)])
            o = si * SLOT
            xy_list = [(v, l)] + [(x, y) for x in range(V) for y in range(L)
                                  if (x, y) != (v, l)]
            for j, (x, y) in enumerate(xy_list):
                X[o + j * S:o + (j + 1) * S] = E[t, x, y, ids_p]
            others = [u for u in range(T) if u != t]
            for oi, u in enumerate(others):
                nk = np.asarray(nid[v, l, oi])
                nk_p = np.concatenate([nk, np.full(KPAD - Kr, nk[0], nk.dtype)])
                X[o + GCOLS + oi * KPAD:o + GCOLS + (oi + 1) * KPAD] = E[u, v, l, nk_p]
            X[o + GCOLS + NK:o + SLOT] = X[o]  # pad columns: dup of row 0
            # within-type mask: valid co-anchor and not the same sample
            MI[:, si * S:si * S + Sr] = 0.0
            MI[np.arange(S), si * S + np.arange(S)] = NEG_BIG
            # cross-type mask: valid negative columns
            for oi in range(3):
                MC[:, si * NK + oi * KPAD:si * NK + oi * KPAD + Kr] = 0.0
            MS[:Sr, si] = 1.0
        in_maps.append({
            "xt": np.ascontiguousarray(X.T),
            "w1t": w1t, "w2t": w2t, "bb": bbm,
            "mi": MI, "mc": MC, "ms": MS,
        })
    return in_maps


def run_on_hw(in_maps, reps=1):
    nc = _get_nc(reps)
    return run_bass_kernel_spmd(nc, in_maps, core_ids=list(range(NCORES)))


def kernel(node_embeddings, W1, b1, W2, b2, idx_prio, idx_rest,
           neg_idx_prio, neg_idx_rest, num_views=2, num_layers=3):
    in_maps = make_in_maps(node_embeddings, W1, b1, W2, b2, idx_prio, idx_rest,
                           neg_idx_prio, neg_idx_rest)
    res = run_on_hw(in_maps)
    _CACHE["last_results"] = res
    total = sum(float(res.results[c]["out"][0, 0]) for c in range(NCORES))
    return np.float32(total / COUNT)


# revision 8
# speedup vs baseline: 46.5571x; 46.5571x over previous
"""Trainium2 Bass kernel for the ContrastiveLearningModule loss.

Math (mirrors the reference):
  P = l2norm(relu(E @ W1.T + b1) @ W2.T + b2)  rowwise over [T,V,L,N,D]
  for each node type t, anchors idx[t][v,l,:]:
    pos  = sum_{(x,y) != (v,l)} exp(z . P[t,x,y,id]/TEMP)
    negi = sum_{s' != s}        exp(z . z_{s'}   /TEMP)
    negc = sum_{o,k}            exp(z . P[o,v,l,nid]/TEMP)
    loss = log(pos+negi+negc) - log(pos);  out = sum(loss)/1440

Key optimization: only the gathered rows (~10.8k of 96k) are ever projected.
The host shards by gathering each core's rows (3 of the 24 (t,v,l) anchor
groups per core, padded to a uniform shape so all 8 cores run one SPMD
program), pre-transposed to [D, cols] so the on-device matmuls need no
transposes.  Each core returns a partial loss sum; the host combines.
"""

import sys

import numpy as np

sys.path.insert(0, "/opt/trn_rl_repo")

import concourse.bacc as bacc
import concourse.bass as bass
import concourse.mybir as mybir
import concourse.tile as tile
from concourse.bass_utils import run_bass_kernel_spmd

F32 = mybir.dt.float32
AF = mybir.ActivationFunctionType

# Problem constants (hardcoded per harness contract).
T, V, L, N, D = 4, 2, 3, 4000, 256
TEMP = 0.5
S = 100          # padded anchors per group (prio groups are exactly 100)
KPAD = 50        # padded cross-negatives per other-type (prio exactly 50)
NK = 3 * KPAD    # 150 cross-negative columns per group
XY = V * L       # 6 (view, layer) slabs
GCOLS = XY * S   # 600 gathered positive columns per group
SLOT = 768       # column stride per group slot (600 + 150 + 18 pad)
NSLOT = 3        # groups per core
NCOL = SLOT * NSLOT  # 2304 packed columns per core
NCORES = 8
COUNT = 1440.0   # total anchor count in the reference loss
NEG_BIG = -15000.0  # additive mask; exp(2*(sims+NEG_BIG)) underflows to 0
COL_BLOCKS = [(0, 512), (512, 512), (1024, 512), (1536, 512), (2048, 256)]

_CACHE = {}


def _emit_body(nc, tc, consts, dram, rep):
    """One full loss computation: projection + per-slot similarities."""
    w1, w2, bb, ones_col, ones_row, eps = consts
    xt_d, mi_d, mc_d, ms_d, out_d = dram
    r = f"r{rep}"

    with (
        tc.tile_pool(name=f"phat{r}", bufs=1) as ppool,
    ):
        # P-hat: normalized projections, [d, col] layout, d split in 2.
        ph = [ppool.tile([128, NCOL], F32, name=f"ph{j}{r}", tag=f"ph{j}")
              for j in (0, 1)]

        # ---- Projection + normalization, 512-column blocks ----
        with (
            tc.tile_pool(name=f"xin{r}", bufs=3) as xpool,
            tc.tile_pool(name=f"work{r}", bufs=2) as wpool,
            tc.tile_pool(name=f"psmm{r}", bufs=4, space=bass.MemorySpace.PSUM) as psmm,
            tc.tile_pool(name=f"psnb{r}", bufs=2, space=bass.MemorySpace.PSUM) as psnb,
        ):
            for c0, w in COL_BLOCKS:
                xs = []
                for k in (0, 1):
                    x = xpool.tile([128, w], F32, name=f"x{k}", tag=f"x{k}")
                    nc.sync.dma_start(x[:], xt_d[k * 128:(k + 1) * 128, c0:c0 + w])
                    xs.append(x)
                hs = []
                for jt in (0, 1):
                    psh = psmm.tile([128, w], F32, name="ps", tag="ps")
                    nc.tensor.matmul(psh[:], w1[0][:, jt * 128:(jt + 1) * 128],
                                     xs[0][:], start=True, stop=False)
                    nc.tensor.matmul(psh[:], w1[1][:, jt * 128:(jt + 1) * 128],
                                     xs[1][:], start=False, stop=True)
                    h = wpool.tile([128, w], F32, name=f"h{jt}", tag=f"h{jt}")
                    nc.scalar.activation(h[:], psh[:], AF.Relu, bias=bb[:, jt:jt + 1])
                    hs.append(h)
                zs = []
                sqs = []
                for jt in (0, 1):
                    psz = psmm.tile([128, w], F32, name="ps", tag="ps")
                    nc.tensor.matmul(psz[:], w2[0][:, jt * 128:(jt + 1) * 128],
                                     hs[0][:], start=True, stop=False)
                    nc.tensor.matmul(psz[:], w2[1][:, jt * 128:(jt + 1) * 128],
                                     hs[1][:], start=False, stop=True)
                    z = wpool.tile([128, w], F32, name=f"z{jt}", tag=f"z{jt}")
                    nc.scalar.activation(z[:], psz[:], AF.Identity,
                                         bias=bb[:, 2 + jt:3 + jt])
                    zs.append(z)
                    sq = wpool.tile([128, w], F32, name=f"sq{jt}", tag=f"sq{jt}")
                    nc.vector.tensor_mul(sq[:], z[:], z[:])
                    sqs.append(sq)
                # ||z||^2 per column via ones-matmul partition reduction
                psn = psnb.tile([1, w], F32, name="psn", tag="psn")
                nc.tensor.matmul(psn[:], ones_col[:], sqs[0][:], start=True, stop=False)
                nc.tensor.matmul(psn[:], ones_col[:], sqs[1][:], start=False, stop=True)
                nrm = wpool.tile([1, w], F32, name="nrm", tag="nrm")
                nc.scalar.activation(nrm[:], psn[:], AF.Sqrt, bias=eps[:])
                rn = wpool.tile([1, w], F32, name="rn", tag="rn")
                nc.vector.reciprocal(rn[:], nrm[:])
                # broadcast 1/||z|| across partitions via rank-1 matmul
                psb = psnb.tile([128, w], F32, name="psb", tag="psb")
                nc.tensor.matmul(psb[:], ones_row[:], rn[:], start=True, stop=True)
                for jt in (0, 1):
                    nc.vector.tensor_mul(ph[jt][:, c0:c0 + w], zs[jt][:], psb[:])

        # ---- Per-slot similarities + loss ----
        with (
            tc.tile_pool(name=f"mpool{r}", bufs=1) as mpool,
            tc.tile_pool(name=f"sbs{r}", bufs=2) as spool,
            tc.tile_pool(name=f"pss{r}", bufs=2, space=bass.MemorySpace.PSUM) as pssim,
            tc.tile_pool(name=f"pst{r}", bufs=1, space=bass.MemorySpace.PSUM) as pstot,
        ):
            mi = mpool.tile([S, NSLOT * S], F32, name="mi", tag="mi")
            nc.sync.dma_start(mi[:], mi_d[:])
            mcm = mpool.tile([S, NSLOT * NK], F32, name="mc", tag="mc")
            nc.sync.dma_start(mcm[:], mc_d[:])
            ms = mpool.tile([S, NSLOT], F32, name="ms", tag="ms")
            nc.sync.dma_start(ms[:], ms_d[:])

            tot = pstot.tile([1, NSLOT], F32, name="tot", tag="tot")
            for s in range(NSLOT):
                o = s * SLOT
                # within-type sims: Z^T Z  [S, S]
                pin = pssim.tile([S, S], F32, name="pin", tag="pin")
                nc.tensor.matmul(pin[:], ph[0][:, o:o + S], ph[0][:, o:o + S],
                                 start=True, stop=False)
                nc.tensor.matmul(pin[:], ph[1][:, o:o + S], ph[1][:, o:o + S],
                                 start=False, stop=True)
                mski = spool.tile([S, S], F32, name="mski", tag="mski")
                nc.vector.tensor_add(mski[:], pin[:], mi[:, s * S:(s + 1) * S])
                ein = spool.tile([S, S], F32, name="ein", tag="ein")
                negin = spool.tile([S, 1], F32, name="negin", tag="negin")
                nc.scalar.activation(ein[:], mski[:], AF.Exp, scale=2.0,
                                     accum_out=negin[:])
                # cross-type sims: Z^T Nmat  [S, NK]
                pc = pssim.tile([S, NK], F32, name="pc", tag="pc")
                nc.tensor.matmul(pc[:], ph[0][:, o:o + S],
                                 ph[0][:, o + GCOLS:o + GCOLS + NK],
                                 start=True, stop=False)
                nc.tensor.matmul(pc[:], ph[1][:, o:o + S],
                                 ph[1][:, o + GCOLS:o + GCOLS + NK],
                                 start=False, stop=True)
                mskc = spool.tile([S, NK], F32, name="mskc", tag="mskc")
                nc.vector.tensor_add(mskc[:], pc[:], mcm[:, s * NK:(s + 1) * NK])
                ec = spool.tile([S, NK], F32, name="ec", tag="ec")
                negc = spool.tile([S, 1], F32, name="negc", tag="negc")
                nc.scalar.activation(ec[:], mskc[:], AF.Exp, scale=2.0,
                                     accum_out=negc[:])
                # positives: per-anchor dot with same node at other (x,y)
                ppos = pssim.tile([S, XY - 1], F32, name="ppos", tag="ppos")
                for xy in range(1, XY):
                    for jt in (0, 1):
                        pr = spool.tile([128, S], F32, name="pr", tag="pr")
                        nc.vector.tensor_mul(pr[:], ph[jt][:, o + xy * S:o + (xy + 1) * S],
                                             ph[jt][:, o:o + S])
                        nc.tensor.matmul(ppos[:, xy - 1:xy], pr[:], ones_col[:],
                                         start=(jt == 0), stop=(jt == 1))
                epos = spool.tile([S, XY - 1], F32, name="epos", tag="epos")
                pos = spool.tile([S, 1], F32, name="pos", tag="pos")
                nc.scalar.activation(epos[:], ppos[:], AF.Exp, scale=2.0,
                                     accum_out=pos[:])
                # loss_s = ln(pos+neg) - ln(pos), then mask+sum via matmul
                neg = spool.tile([S, 1], F32, name="neg", tag="neg")
                nc.vector.tensor_add(neg[:], negin[:], negc[:])
                den = spool.tile([S, 1], F32, name="den", tag="den")
                nc.vector.tensor_add(den[:], neg[:], pos[:])
                lnden = spool.tile([S, 1], F32, name="lnden", tag="lnden")
                nc.scalar.activation(lnden[:], den[:], AF.Ln)
                lnpos = spool.tile([S, 1], F32, name="lnpos", tag="lnpos")
                nc.scalar.activation(lnpos[:], pos[:], AF.Ln)
                lossv = spool.tile([S, 1], F32, name="lossv", tag="lossv")
                nc.vector.tensor_sub(lossv[:], lnden[:], lnpos[:])
                nc.tensor.matmul(tot[:, s:s + 1], lossv[:], ms[:, s:s + 1],
                                 start=True, stop=True)
            osb = spool.tile([1, 1], F32, name="osb", tag="osb")
            nc.vector.reduce_sum(osb[:], tot[:], axis=mybir.AxisListType.X)
            nc.sync.dma_start(out_d[:], osb[:])


def _build_nc(reps=1):
    nc = bacc.Bacc("TRN2", target_bir_lowering=False, debug=False)

    xt_d = nc.dram_tensor("xt", [D, NCOL], F32, kind="ExternalInput")
    w1t_d = nc.dram_tensor("w1t", [D, D], F32, kind="ExternalInput")
    w2t_d = nc.dram_tensor("w2t", [D, D], F32, kind="ExternalInput")
    bb_d = nc.dram_tensor("bb", [128, 4], F32, kind="ExternalInput")
    mi_d = nc.dram_tensor("mi", [S, NSLOT * S], F32, kind="ExternalInput")
    mc_d = nc.dram_tensor("mc", [S, NSLOT * NK], F32, kind="ExternalInput")
    ms_d = nc.dram_tensor("ms", [S, NSLOT], F32, kind="ExternalInput")
    out_d = nc.dram_tensor("out", [1, 1], F32, kind="ExternalOutput")

    with tile.TileContext(nc) as tc:
        with tc.tile_pool(name="const", bufs=1) as cpool:
            w1 = [cpool.tile([128, D], F32, name=f"w1_{k}", tag=f"w1_{k}")
                  for k in (0, 1)]
            w2 = [cpool.tile([128, D], F32, name=f"w2_{k}", tag=f"w2_{k}")
                  for k in (0, 1)]
            for k in (0, 1):
                nc.sync.dma_start(w1[k][:], w1t_d[k * 128:(k + 1) * 128, :])
                nc.sync.dma_start(w2[k][:], w2t_d[k * 128:(k + 1) * 128, :])
            bb = cpool.tile([128, 4], F32, name="bb", tag="bb")
            nc.sync.dma_start(bb[:], bb_d[:])
            ones_col = cpool.tile([128, 1], F32, name="ones_col", tag="ones_col")
            nc.vector.memset(ones_col[:], 1.0)
            ones_row = cpool.tile([1, 128], F32, name="ones_row", tag="ones_row")
            nc.vector.memset(ones_row[:], 1.0)
            eps = cpool.tile([1, 1], F32, name="eps", tag="eps")
            nc.vector.memset(eps[:], 1e-24)

            consts = (w1, w2, bb, ones_col, ones_row, eps)
            dram = (xt_d, mi_d, mc_d, ms_d, out_d)
            for rep in range(reps):
                _emit_body(nc, tc, consts, dram, rep)

    nc.compile()
    return nc


def _get_nc(reps=1):
    key = ("nc", reps)
    if key not in _CACHE:
        _CACHE[key] = _build_nc(reps)
    return _CACHE[key]


def _groups():
    gs = [(t, v, l) for t in range(T) for v in range(V) for l in range(L)]
    return [[gs[c], gs[c + NCORES], gs[c + 2 * NCORES]] for c in range(NCORES)]


def make_in_maps(node_embeddings, W1, b1, W2, b2, idx_prio, idx_rest,
                 neg_idx_prio, neg_idx_rest):
    E = np.asarray(node_embeddings, dtype=np.float32)
    W1 = np.asarray(W1, dtype=np.float32)
    b1 = np.asarray(b1, dtype=np.float32)
    W2 = np.asarray(W2, dtype=np.float32)
    b2 = np.asarray(b2, dtype=np.float32)
    idxp = np.asarray(idx_prio)
    idxr = np.asarray(idx_rest)
    nidxp = np.asarray(neg_idx_prio)
    nidxr = np.asarray(neg_idx_rest)

    w1t = np.ascontiguousarray(W1.T)
    w2t = np.ascontiguousarray(W2.T)
    bbm = np.stack([b1[:128], b1[128:], b2[:128], b2[128:]], axis=1)
    bbm = np.ascontiguousarray(bbm, dtype=np.float32)

    in_maps = []
    for gs in _groups():
        X = np.empty((NCOL, D), np.float32)
        MI = np.full((S, NSLOT * S), NEG_BIG, np.float32)
        MC = np.full((S, NSLOT * NK), NEG_BIG, np.float32)
        MS = np.zeros((S, NSLOT), np.float32)
        for si, (t, v, l) in enumerate(gs):
            if t < 2:
                idx, nid, Sr, Kr = idxp[t], nidxp[t], 100, 50
            else:
                idx, nid, Sr, Kr = idxr[t - 2], nidxr[t - 2], 20, 10
            ids = np.asarray(idx[v, l])
            ids_p = np.concatenate([ids, np.full(S - Sr, ids[0], ids.dtype)])
            o = si * SLOT
            xy_list = [(v, l)] + [(x, y) for x in range(V) for y in range(L)
                                  if (x, y) != (v, l)]
            for j, (x, y) in enumerate(xy_list):
                X[o + j * S:o + (j + 1) * S] = E[t, x, y, ids_p]
            others = [u for u in range(T) if u != t]
            for oi, u in enumerate(others):
                nk = np.asarray(nid[v, l, oi])
                nk_p = np.concatenate([nk, np.full(KPAD - Kr, nk[0], nk.dtype)])
                X[o + GCOLS + oi * KPAD:o + GCOLS + (oi + 1) * KPAD] = E[u, v, l, nk_p]
            X[o + GCOLS + NK:o + SLOT] = X[o]  # pad columns: dup of row 0
            # within-type mask: valid co-anchor and not the same sample
            MI[:, si * S:si * S + Sr] = 0.0
            MI[np.arange(S), si * S + np.arange(S)] = NEG_BIG
            # cross-type mask: valid negative columns
            for oi in range(3):
                MC[:, si * NK + oi * KPAD:si * NK + oi * KPAD + Kr] = 0.0
            MS[:Sr, si] = 1.0
        in_maps.append({
            "xt": np.ascontiguousarray(X.T),
            "w1t": w1t, "w2t": w2t, "bb": bbm,
            "mi": MI, "mc": MC, "ms": MS,
        })
    return in_maps


def _make_runner(nc):
    """Lower nc to a cached jitted SPMD executable (mirrors
    bass2jax.run_bass_via_pjrt, but reusable across calls so repeat
    executions skip tracing/compilation)."""
    import jax
    from jax.experimental.shard_map import shard_map
    from jax.sharding import Mesh, PartitionSpec

    from concourse import bass2jax
    from concourse import mybir as mb

    bass2jax.install_neuronx_cc_hook()
    partition_name = (nc.partition_id_tensor.name
                      if nc.partition_id_tensor else None)
    in_names, out_names, out_avals = [], [], []
    for alloc in nc.m.functions[0].allocations:
        if not isinstance(alloc, mb.MemoryLocationSet):
            continue
        name = alloc.memorylocations[0].name
        if alloc.kind == "ExternalInput":
            if name != partition_name:
                in_names.append(name)
        elif alloc.kind == "ExternalOutput":
            out_names.append(name)
            out_avals.append(jax.core.ShapedArray(
                tuple(alloc.tensor_shape), mb.dt.np(alloc.dtype)))
    n_params = len(in_names)
    n_outs = len(out_avals)
    all_in_names = list(in_names) + list(out_names)
    if partition_name is not None:
        all_in_names.append(partition_name)

    def _body(*args):
        operands = list(args)
        if partition_name is not None:
            operands.append(bass2jax.partition_id_tensor())
        return tuple(bass2jax._bass_exec_p.bind(
            *operands,
            out_avals=tuple(out_avals),
            in_names=tuple(all_in_names),
            out_names=tuple(out_names),
            lowering_input_output_aliases=(),
            sim_require_finite=True,
            sim_require_nnan=True,
            nc=nc,
        ))

    devices = jax.devices()[:NCORES]
    mesh = Mesh(np.asarray(devices), ("core",))
    donate = tuple(range(n_params, n_params + n_outs))
    sharded = jax.jit(
        shard_map(_body, mesh=mesh,
                  in_specs=(PartitionSpec("core"),) * (n_params + n_outs),
                  out_specs=(PartitionSpec("core"),) * n_outs,
                  check_rep=False),
        donate_argnums=donate, keep_unused=True)

    def run(in_maps, device_inputs=None):
        if device_inputs is None:
            device_inputs = [
                np.concatenate([np.asarray(m[name]) for m in in_maps], axis=0)
                for name in in_names]
        zeros = [np.zeros((NCORES * a.shape[0], *a.shape[1:]), a.dtype)
                 for a in out_avals]
        out_arrs = sharded(*device_inputs, *zeros)
        return [
            {name: np.asarray(out_arrs[i]).reshape(NCORES, *out_avals[i].shape)[c]
             for i, name in enumerate(out_names)}
            for c in range(NCORES)
        ]

    run.in_names = in_names
    return run


def _get_runner(reps=1):
    key = ("runner", reps)
    if key not in _CACHE:
        _CACHE[key] = _make_runner(_get_nc(reps))
    return _CACHE[key]


class _Res:
    def __init__(self, results):
        self.results = results


def run_on_hw(in_maps, reps=1, device_inputs=None):
    runner = _get_runner(reps)
    return _Res(runner(in_maps, device_inputs=device_inputs))


def kernel(node_embeddings, W1, b1, W2, b2, idx_prio, idx_rest,
           neg_idx_prio, neg_idx_rest, num_views=2, num_layers=3):
    in_maps = make_in_maps(node_embeddings, W1, b1, W2, b2, idx_prio, idx_rest,
                           neg_idx_prio, neg_idx_rest)
    res = run_on_hw(in_maps)
    _CACHE["last_results"] = res
    total = sum(float(res.results[c]["out"][0, 0]) for c in range(NCORES))
    return np.float32(total / COUNT)


# revision 38
# speedup vs baseline: 531.6983x; 11.4203x over previous
"""Trainium2 Bass kernel for the ContrastiveLearningModule loss.

Math (mirrors the reference):
  P = l2norm(relu(E @ W1.T + b1) @ W2.T + b2)  rowwise over [T,V,L,N,D]
  for each node type t, anchors idx[t][v,l,:]:
    pos  = sum_{(x,y) != (v,l)} exp(z . P[t,x,y,id]/TEMP)
    negi = sum_{s' != s}        exp(z . z_{s'}   /TEMP)
    negc = sum_{o,k}            exp(z . P[o,v,l,nid]/TEMP)
    loss = log(pos+negi+negc) - log(pos);  out = sum(loss)/1440

Key optimizations:
 - only the gathered rows (~10.8k of 96k) are ever projected.  The host
   shards by gathering each core's rows (3 of the 24 (t,v,l) anchor groups
   per core, padded to a uniform shape so all 8 cores run one SPMD program),
   pre-transposed to [D, cols] so the on-device matmuls need no transposes.
 - bf16 operands for all large matmuls/elementwise (fp32 PSUM accumulation);
   final rel err ~1e-5.
 - one ACT function table (relu/identity/exp/ln): 1/||z|| = exp(-0.5 ln n2)
   instead of sqrt+reciprocal, avoiding table reloads.
 - masking via host-built additive masks (exp underflows to exact 0).
Each core returns a partial loss sum; the host combines.
"""

import sys

import numpy as np

sys.path.insert(0, "/opt/trn_rl_repo")

import concourse.bacc as bacc
import concourse.bass as bass
import concourse.mybir as mybir
import concourse.tile as tile
from concourse.bass_utils import run_bass_kernel_spmd
from concourse.hw_specs import get_activation_tables as _real_gat

_ONE_TABLE = "natural_log_exp_and_others"  # holds relu/identity/exp/ln/copy


def _gat_one_table(arch):
    """Restrict the act-table-load pass to a single function set that covers
    every ACT func this kernel uses, so exactly one LoadActFuncSet is
    emitted (the greedy per-func chooser otherwise thrashes between the
    exp and ln tables every block).  Table ids stay valid because the
    list order/length is unchanged -- other sets are just emptied."""
    tabs = _real_gat(arch)
    if _ONE_TABLE in tabs:
        return {k: (v if k == _ONE_TABLE else set()) for k, v in tabs.items()}
    return tabs


bacc.get_activation_tables = _gat_one_table

F32 = mybir.dt.float32
BF16 = mybir.dt.bfloat16
AF = mybir.ActivationFunctionType
NP_BF16 = mybir.dt.np(BF16)

# Problem constants (hardcoded per harness contract).
T, V, L, N, D = 4, 2, 3, 4000, 256
TEMP = 0.5
S = 100          # padded anchors per group (prio groups are exactly 100)
KPAD = 50        # padded cross-negatives per other-type (prio exactly 50)
NK = 3 * KPAD    # 150 cross-negative columns per group
XY = V * L       # 6 (view, layer) slabs
GCOLS = XY * S   # 600 gathered positive columns per group
SLOT = 768       # column stride per group slot (600 + 150 + 18 pad)
NSLOT = 3        # groups per core
NCOL = SLOT * NSLOT  # 2304 packed columns per core
NCORES = 8
COUNT = 1440.0   # total anchor count in the reference loss
NEG_BIG = -15000.0  # additive mask; exp(2*(sims+NEG_BIG)) underflows to 0
MMW = 512  # max matmul moving free dim into one PSUM bank

_CACHE = {}


def _emit_body(nc, tc, consts, dram, pools, rep):
    """One full loss computation: per-slot projection + similarities,
    interleaved so slot s's similarity work overlaps slot s+1's
    projection."""
    w1, w2, bb, ones_col, ones_colf, ones_row, ones11, eps, mi, mcm, ms = consts
    xt_d, out_d = dram
    ppool, xpool, wpool, spool, psmm, psnb, pssim = pools
    r = f"r{rep}"
    w = SLOT
    halves = [slice(h0, min(h0 + MMW, w)) for h0 in range(0, w, MMW)]

    if True:
        # masked per-anchor losses, one column per slot; summed at the end
        lm_all = spool.tile([S, NSLOT], F32, name=f"lm{r}", tag="lm")
        for s in range(NSLOT):
            o = s * SLOT
            # ---- Projection + normalization of slot s's 768 columns ----
            ph = [ppool.tile([128, SLOT], BF16, name=f"ph{j}_{s}{r}",
                             tag=f"ph{j}_{s}") for j in (0, 1)]
            xtile = xpool.tile([128, 2 * w], BF16, name="xtile", tag="xtile")
            nc.sync.dma_start(xtile[:], xt_d[:, 2 * o:2 * o + 2 * w])
            xs = [xtile[:, 0:w], xtile[:, w:2 * w]]
            hs = []
            for jt in (0, 1):
                psh = psmm.tile([128, w], F32, name="ps", tag="ps")
                for sl in halves:
                    nc.tensor.matmul(psh[:, sl], w1[0][:, jt * 128:(jt + 1) * 128],
                                     xs[0][:, sl], start=True, stop=False)
                    nc.tensor.matmul(psh[:, sl], w1[1][:, jt * 128:(jt + 1) * 128],
                                     xs[1][:, sl], start=False, stop=True)
                h = wpool.tile([128, w], BF16, name=f"h{jt}", tag=f"h{jt}")
                nc.scalar.activation(h[:], psh[:], AF.Relu, bias=bb[:, jt:jt + 1])
                hs.append(h)
            zs = []
            sqs = []
            for jt in (0, 1):
                psz = psmm.tile([128, w], F32, name="ps", tag="ps")
                for sl in halves:
                    nc.tensor.matmul(psz[:, sl], w2[0][:, jt * 128:(jt + 1) * 128],
                                     hs[0][:, sl], start=True, stop=False)
                    nc.tensor.matmul(psz[:, sl], w2[1][:, jt * 128:(jt + 1) * 128],
                                     hs[1][:, sl], start=False, stop=True)
                # z = psum + b2 on DVE (keeps ACT free for ln/exp)
                z = wpool.tile([128, w], BF16, name=f"z{jt}", tag=f"z{jt}")
                nc.vector.tensor_scalar_add(z[:], psz[:], bb[:, 2 + jt:3 + jt])
                zs.append(z)
                sq = wpool.tile([128, w], BF16, name=f"sq{jt}", tag=f"sq{jt}")
                nc.vector.tensor_mul(sq[:], z[:], z[:])
                sqs.append(sq)
            # ||z||^2 per column via ones-matmul partition reduction
            psn = psnb.tile([1, w], F32, name="psn", tag="pnb")
            for sl in halves:
                nc.tensor.matmul(psn[:, sl], ones_col[:], sqs[0][:, sl],
                                 start=True, stop=False)
                nc.tensor.matmul(psn[:, sl], ones_col[:], sqs[1][:, sl],
                                 start=False, stop=True)
            # 1/||z|| = exp(-0.5 ln(n2)) -- keeps exp/ln/relu in one table
            lnn = wpool.tile([1, w], F32, name="lnn", tag="lnn")
            nc.scalar.activation(lnn[:], psn[:], AF.Ln, bias=eps[:])
            rn = wpool.tile([1, w], BF16, name="rn", tag="rn")
            nc.scalar.activation(rn[:], lnn[:], AF.Exp, scale=-0.5)
            # broadcast 1/||z|| across partitions via rank-1 matmul
            psb = psnb.tile([128, w], F32, name="psb", tag="pnb")
            for sl in halves:
                nc.tensor.matmul(psb[:, sl], ones_row[:], rn[:, sl],
                                 start=True, stop=True)
            for jt in (0, 1):
                nc.vector.tensor_mul(ph[jt][:], zs[jt][:], psb[:])

            # ---- Similarities + loss for slot s ----
            # within-type sims: Z^T Z  [S, S]
            pin = pssim.tile([S, S], F32, name="pin", tag="sim")
            nc.tensor.matmul(pin[:], ph[0][:, 0:S], ph[0][:, 0:S],
                             start=True, stop=False)
            nc.tensor.matmul(pin[:], ph[1][:, 0:S], ph[1][:, 0:S],
                             start=False, stop=True)
            mski = spool.tile([S, S], F32, name="mski", tag="mski")
            nc.vector.tensor_add(mski[:], pin[:], mi[:, s * S:(s + 1) * S])
            ein = spool.tile([S, S], F32, name="ein", tag="ein")
            negin = spool.tile([S, 1], F32, name="negin", tag="negin")
            nc.scalar.activation(ein[:], mski[:], AF.Exp, scale=2.0,
                                 accum_out=negin[:])
            # cross-type sims: Z^T Nmat  [S, NK]
            pc = pssim.tile([S, NK], F32, name="pc", tag="sim")
            nc.tensor.matmul(pc[:], ph[0][:, 0:S], ph[0][:, GCOLS:GCOLS + NK],
                             start=True, stop=False)
            nc.tensor.matmul(pc[:], ph[1][:, 0:S], ph[1][:, GCOLS:GCOLS + NK],
                             start=False, stop=True)
            mskc = spool.tile([S, NK], F32, name="mskc", tag="mskc")
            nc.vector.tensor_add(mskc[:], pc[:], mcm[:, s * NK:(s + 1) * NK])
            ec = spool.tile([S, NK], F32, name="ec", tag="ec")
            negc = spool.tile([S, 1], F32, name="negc", tag="negc")
            nc.scalar.activation(ec[:], mskc[:], AF.Exp, scale=2.0,
                                 accum_out=negc[:])
            # positives: per-anchor dot with same node at other (x,y).
            # PR[:, xy*S+s] = ph[:, (xy+1)*S+s] * Z[:, s]; column-sum via
            # ones-matmul -> [1, 5*S] sims, exp, reduce over xy.
            ppr = pssim.tile([1, (XY - 1) * S], F32, name="ppr", tag="sim")
            for jt in (0, 1):
                pr = spool.tile([128, (XY - 1) * S], BF16, name="pr", tag="pr")
                zb = ph[jt][:, 0:S].unsqueeze(1).to_broadcast([128, XY - 1, S])
                nc.vector.tensor_mul(
                    pr[:].rearrange("p (a b) -> p a b", a=XY - 1),
                    ph[jt][:, S:XY * S].rearrange("p (a b) -> p a b", a=XY - 1),
                    zb)
                nc.tensor.matmul(ppr[:], ones_col[:], pr[:],
                                 start=(jt == 0), stop=(jt == 1))
            epr = spool.tile([1, (XY - 1) * S], F32, name="epr", tag="epr")
            nc.scalar.activation(epr[:], ppr[:], AF.Exp, scale=2.0)
            # pos[s] = sum_xy epr[xy*S+s]: 5 accumulating rank-1 transposes
            psp = pssim.tile([S, 1], F32, name="psp", tag="sim")
            for xy in range(XY - 1):
                nc.tensor.matmul(psp[:], epr[:, xy * S:(xy + 1) * S], ones11[:],
                                 start=(xy == 0), stop=(xy == XY - 2))
            # loss_s = ln(pos+neg) - ln(pos), then mask+sum via matmul
            neg = spool.tile([S, 1], F32, name="neg", tag="neg")
            nc.vector.tensor_add(neg[:], negin[:], negc[:])
            den = spool.tile([S, 1], F32, name="den", tag="den")
            nc.vector.tensor_add(den[:], neg[:], psp[:])
            lnden = spool.tile([S, 1], F32, name="lnden", tag="lnden")
            nc.scalar.activation(lnden[:], den[:], AF.Ln)
            lnpos = spool.tile([S, 1], F32, name="lnpos", tag="lnpos")
            nc.scalar.activation(lnpos[:], psp[:], AF.Ln)
            lossv = spool.tile([S, 1], F32, name="lossv", tag="lossv")
            nc.vector.tensor_sub(lossv[:], lnden[:], lnpos[:])
            nc.vector.tensor_mul(lm_all[:, s:s + 1], lossv[:], ms[:, s:s + 1])
        tot = pssim.tile([1, NSLOT], F32, name="tot", tag="sim")
        nc.tensor.matmul(tot[:], ones_colf[:S, :], lm_all[:], start=True, stop=True)
        osb = spool.tile([1, 1], F32, name="osb", tag="osb")
        nc.vector.reduce_sum(osb[:], tot[:], axis=mybir.AxisListType.X)
        nc.sync.dma_start(out_d[:], osb[:])


def _build_nc(reps=1, loop_iters=None):
    nc = bacc.Bacc("TRN2", target_bir_lowering=False, debug=False)

    xt_d = nc.dram_tensor("xt", [128, 2 * NCOL], BF16, kind="ExternalInput")
    wp_d = nc.dram_tensor("wp", [128, 4 * D], BF16, kind="ExternalInput")
    bb_d = nc.dram_tensor("bb", [128, 4], F32, kind="ExternalInput")
    mk_d = nc.dram_tensor("mk", [S, NSLOT * (S + NK + 1)], F32, kind="ExternalInput")
    out_d = nc.dram_tensor("out", [1, 1], F32, kind="ExternalOutput")

    with tile.TileContext(nc) as tc:
        with tc.tile_pool(name="const", bufs=1) as cpool:
            wtile = cpool.tile([128, 4 * D], BF16, name="wtile", tag="wtile")
            nc.sync.dma_start(wtile[:], wp_d[:])
            w1 = [wtile[:, 0:D], wtile[:, D:2 * D]]
            w2 = [wtile[:, 2 * D:3 * D], wtile[:, 3 * D:4 * D]]
            bb = cpool.tile([128, 4], F32, name="bb", tag="bb")
            nc.sync.dma_start(bb[:], bb_d[:])
            ones_col = cpool.tile([128, 1], BF16, name="ones_col", tag="ones_col")
            nc.vector.memset(ones_col[:], 1.0)
            ones_colf = cpool.tile([128, 1], F32, name="ones_colf", tag="ones_colf")
            nc.vector.memset(ones_colf[:], 1.0)
            ones_row = cpool.tile([1, 128], BF16, name="ones_row", tag="ones_row")
            nc.vector.memset(ones_row[:], 1.0)
            ones11 = cpool.tile([1, 1], F32, name="ones11", tag="ones11")
            nc.vector.memset(ones11[:], 1.0)
            eps = cpool.tile([1, 1], F32, name="eps", tag="eps")
            nc.vector.memset(eps[:], 1e-24)
            mk = cpool.tile([S, NSLOT * (S + NK + 1)], F32, name="mk", tag="mk")
            nc.sync.dma_start(mk[:], mk_d[:])
            mi = mk[:, 0:NSLOT * S]
            mcm = mk[:, NSLOT * S:NSLOT * (S + NK)]
            ms = mk[:, NSLOT * (S + NK):]

            consts = (w1, w2, bb, ones_col, ones_colf, ones_row, ones11, eps,
                      mi, mcm, ms)
            dram = (xt_d, out_d)
            with (
                tc.tile_pool(name="phat", bufs=2) as ppool,
                tc.tile_pool(name="xin", bufs=3) as xpool,
                tc.tile_pool(name="work", bufs=2) as wpool,
                tc.tile_pool(name="sbs", bufs=2) as spool,
                tc.tile_pool(name="psmm", bufs=2, space=bass.MemorySpace.PSUM) as psmm,
                tc.tile_pool(name="psnb", bufs=1, space=bass.MemorySpace.PSUM) as psnb,
                tc.tile_pool(name="pss", bufs=2, space=bass.MemorySpace.PSUM) as pssim,
            ):
                pools = (ppool, xpool, wpool, spool, psmm, psnb, pssim)
                if loop_iters is not None:
                    # device-side loop for wall-clock benchmarking
                    with tc.For_i(0, loop_iters, 1,
                                  hint_engines=(mybir.EngineType.PE,
                                                mybir.EngineType.DVE,
                                                mybir.EngineType.Activation)):
                        _emit_body(nc, tc, consts, dram, pools, 0)
                else:
                    for rep in range(reps):
                        _emit_body(nc, tc, consts, dram, pools, rep)

    nc.compile()
    return nc


def _get_nc(reps=1, loop_iters=None):
    key = ("nc", reps, loop_iters)
    if key not in _CACHE:
        _CACHE[key] = _build_nc(reps, loop_iters)
    return _CACHE[key]


def _groups():
    gs = [(t, v, l) for t in range(T) for v in range(V) for l in range(L)]
    return [[gs[c], gs[c + NCORES], gs[c + 2 * NCORES]] for c in range(NCORES)]


def make_in_maps(node_embeddings, W1, b1, W2, b2, idx_prio, idx_rest,
                 neg_idx_prio, neg_idx_rest):
    E = np.asarray(node_embeddings, dtype=np.float32)
    W1 = np.asarray(W1, dtype=np.float32)
    b1 = np.asarray(b1, dtype=np.float32)
    W2 = np.asarray(W2, dtype=np.float32)
    b2 = np.asarray(b2, dtype=np.float32)
    idxp = np.asarray(idx_prio)
    idxr = np.asarray(idx_rest)
    nidxp = np.asarray(neg_idx_prio)
    nidxr = np.asarray(neg_idx_rest)

    w1t = W1.T
    w2t = W2.T
    # packed weights: [128, 4D] = [w1t rows 0:128 | w1t rows 128:256 |
    #                              w2t rows 0:128 | w2t rows 128:256]
    wp = np.concatenate([w1t[:128], w1t[128:], w2t[:128], w2t[128:]],
                        axis=1).astype(NP_BF16)
    wp = np.ascontiguousarray(wp)
    bbm = np.stack([b1[:128], b1[128:], b2[:128], b2[128:]], axis=1)
    bbm = np.ascontiguousarray(bbm, dtype=np.float32)

    in_maps = []
    for gs in _groups():
        X = np.empty((NCOL, D), np.float32)
        MI = np.full((S, NSLOT * S), NEG_BIG, np.float32)
        MC = np.full((S, NSLOT * NK), NEG_BIG, np.float32)
        MS = np.zeros((S, NSLOT), np.float32)
        for si, (t, v, l) in enumerate(gs):
            if t < 2:
                idx, nid, Sr, Kr = idxp[t], nidxp[t], 100, 50
            else:
                idx, nid, Sr, Kr = idxr[t - 2], nidxr[t - 2], 20, 10
            ids = np.asarray(idx[v, l])
            ids_p = np.concatenate([ids, np.full(S - Sr, ids[0], ids.dtype)])
            o = si * SLOT
            xy_list = [(v, l)] + [(x, y) for x in range(V) for y in range(L)
                                  if (x, y) != (v, l)]
            for j, (x, y) in enumerate(xy_list):
                X[o + j * S:o + (j + 1) * S] = E[t, x, y, ids_p]
            others = [u for u in range(T) if u != t]
            for oi, u in enumerate(others):
                nk = np.asarray(nid[v, l, oi])
                nk_p = np.concatenate([nk, np.full(KPAD - Kr, nk[0], nk.dtype)])
                X[o + GCOLS + oi * KPAD:o + GCOLS + (oi + 1) * KPAD] = E[u, v, l, nk_p]
            X[o + GCOLS + NK:o + SLOT] = X[o]  # pad columns: dup of row 0
            # within-type mask: valid co-anchor and not the same sample
            MI[:, si * S:si * S + Sr] = 0.0
            MI[np.arange(S), si * S + np.arange(S)] = NEG_BIG
            # cross-type mask: valid negative columns
            for oi in range(3):
                MC[:, si * NK + oi * KPAD:si * NK + oi * KPAD + Kr] = 0.0
            MS[:Sr, si] = 1.0
        # xt packed per slot: [128, 2*NCOL], cols s*1536+j*768+c =
        # X.T[j*128+p, s*768+c]
        XT = X.T.astype(NP_BF16)
        XP = np.empty((128, 2 * NCOL), NP_BF16)
        for s in range(NSLOT):
            for j in (0, 1):
                XP[:, s * 2 * SLOT + j * SLOT:(s * 2 + j + 1) * SLOT] = \
                    XT[j * 128:(j + 1) * 128, s * SLOT:(s + 1) * SLOT]
        MK = np.concatenate([MI, MC, MS], axis=1)
        in_maps.append({
            "xt": np.ascontiguousarray(XP),
            "wp": wp, "bb": bbm,
            "mk": np.ascontiguousarray(MK),
        })
    return in_maps


def _make_runner(nc):
    """Lower nc to a cached jitted SPMD executable (mirrors
    bass2jax.run_bass_via_pjrt, but reusable across calls so repeat
    executions skip tracing/compilation)."""
    import jax
    from jax.experimental.shard_map import shard_map
    from jax.sharding import Mesh, PartitionSpec

    from concourse import bass2jax
    from concourse import mybir as mb

    bass2jax.install_neuronx_cc_hook()
    partition_name = (nc.partition_id_tensor.name
                      if nc.partition_id_tensor else None)
    in_names, out_names, out_avals = [], [], []
    for alloc in nc.m.functions[0].allocations:
        if not isinstance(alloc, mb.MemoryLocationSet):
            continue
        name = alloc.memorylocations[0].name
        if alloc.kind == "ExternalInput":
            if name != partition_name:
                in_names.append(name)
        elif alloc.kind == "ExternalOutput":
            out_names.append(name)
            out_avals.append(jax.core.ShapedArray(
                tuple(alloc.tensor_shape), mb.dt.np(alloc.dtype)))
    n_params = len(in_names)
    n_outs = len(out_avals)
    all_in_names = list(in_names) + list(out_names)
    if partition_name is not None:
        all_in_names.append(partition_name)

    def _body(*args):
        operands = list(args)
        if partition_name is not None:
            operands.append(bass2jax.partition_id_tensor())
        return tuple(bass2jax._bass_exec_p.bind(
            *operands,
            out_avals=tuple(out_avals),
            in_names=tuple(all_in_names),
            out_names=tuple(out_names),
            lowering_input_output_aliases=(),
            sim_require_finite=True,
            sim_require_nnan=True,
            nc=nc,
        ))

    devices = jax.devices()[:NCORES]
    mesh = Mesh(np.asarray(devices), ("core",))
    donate = tuple(range(n_params, n_params + n_outs))
    sharded = jax.jit(
        shard_map(_body, mesh=mesh,
                  in_specs=(PartitionSpec("core"),) * (n_params + n_outs),
                  out_specs=(PartitionSpec("core"),) * n_outs,
                  check_rep=False),
        donate_argnums=donate, keep_unused=True)

    def run(in_maps, device_inputs=None):
        if device_inputs is None:
            device_inputs = [
                np.concatenate([np.asarray(m[name]) for m in in_maps], axis=0)
                for name in in_names]
        zeros = [np.zeros((NCORES * a.shape[0], *a.shape[1:]), a.dtype)
                 for a in out_avals]
        out_arrs = sharded(*device_inputs, *zeros)
        return [
            {name: np.asarray(out_arrs[i]).reshape(NCORES, *out_avals[i].shape)[c]
             for i, name in enumerate(out_names)}
            for c in range(NCORES)
        ]

    run.in_names = in_names
    run.mesh = mesh
    return run


def _get_runner(reps=1, loop_iters=None):
    key = ("runner", reps, loop_iters)
    if key not in _CACHE:
        _CACHE[key] = _make_runner(_get_nc(reps, loop_iters))
    return _CACHE[key]


class _Res:
    def __init__(self, results):
        self.results = results


def run_on_hw(in_maps, reps=1, device_inputs=None, loop_iters=None):
    runner = _get_runner(reps, loop_iters)
    return _Res(runner(in_maps, device_inputs=device_inputs))


def kernel(node_embeddings, W1, b1, W2, b2, idx_prio, idx_rest,
           neg_idx_prio, neg_idx_rest, num_views=2, num_layers=3):
    in_maps = make_in_maps(node_embeddings, W1, b1, W2, b2, idx_prio, idx_rest,
                           neg_idx_prio, neg_idx_rest)
    res = run_on_hw(in_maps)
    _CACHE["last_results"] = res
    total = sum(float(res.results[c]["out"][0, 0]) for c in range(NCORES))
    return np.float32(total / COUNT)
